# revision 22
# baseline (speedup 1.0000x reference)
# Distributed Bass kernel for GBottleneck GNN (Pixel2Mesh GConv stack) on 8
# TRN2 NeuronCores.
#
# Reference computation (per layer):  Z = adj @ (h @ W) + h @ Wl + b
#   x: [8192, 256], adj: [8192, 8192], hidden 192, 6 residual blocks of 2
#   gconvs, final output conv to 3 dims. 14 gconvs total.
#
# Sharding: node dim N=8192 row-sharded across 8 cores (1024 rows each of
# adj/x/h). Weights replicated. Per gconv, each core computes its local
# support rows S_loc = h_loc @ W [1024, 192], all-gathers S (bf16), and
# contracts its adj rows against the full support.
#
# Key layout decisions:
#  - adj shard is cast fp32->bf16 once (SWDGE cast DMA, HBM->HBM), then
#    DMA-transposed (XBAR) into SBUF where it stays RESIDENT for all 14
#    gconvs: adjT [k=8192 (64 tiles of 128 partitions), j=1024] bf16, 16MB.
#    adj is read from HBM exactly once.
#  - Activations are kept transposed: Ht [192, 1024] (fp32 carry + bf16
#    matmul copy). In this orientation the adj contraction, the Wl path and
#    the bias+relu eviction all happen in PSUM [m', j] with zero extra
#    transposes; S_loc = h @ W comes out j-major which is exactly what the
#    all-gather concatenation needs.
#  - Big matmul: stationary = S_full[k_tile, m'] (bf16), moving = adjT
#    [k_tile, j], accumulating Z^T [192, 1024] in PSUM over 64 k tiles.
#    The h @ Wl path accumulates into the same PSUM banks first (it only
#    depends on local data, so it overlaps the AllGather latency).
#  - fp32 PSUM accumulation everywhere; residual carry in fp32.

import os
import sys

import numpy as np

if "/opt/trn_rl_repo" not in sys.path:
    sys.path.insert(0, "/opt/trn_rl_repo")

N, IN_DIM, HID, OUT_DIM, L = 8192, 256, 192, 3, 6
R = 8  # cores

TRACE = False  # test.py sets this for profiled runs
LAST_RESULT = None  # BassKernelResults of the most recent run

_cache = {}


def build_kernel(n=N, debug=False):
    import concourse.bass as bass
    import concourse.mybir as mybir
    import concourse.tile as tile
    from concourse import bacc
    from concourse.masks import make_identity

    f32 = mybir.dt.float32
    bf16 = mybir.dt.bfloat16
    AF = mybir.ActivationFunctionType

    nloc = n // R
    KT = n // 128          # global k tiles
    JT = nloc // 128       # local j tiles
    assert JT >= 2 and JT % 2 == 0, "need an even number of local j tiles"
    JB = 512 if nloc >= 512 else nloc  # moving-operand j block
    NJB = nloc // JB
    CH = min(8, KT)        # k-tiles per S-chunk DMA
    NCH = KT // CH
    ACH = min(4, KT)       # k-tiles per adj cast chunk (column chunk)

    nc = bacc.Bacc("TRN2", target_bir_lowering=False, debug=debug, num_devices=R)

    # ---------------- I/O ----------------
    # x and adj arrive HOST-TRANSPOSED (layout prep done in kernel() while
    # sharding): xT = x_loc.T [IN_DIM, nloc]; adjT = adj_loc.T with k-tiles
    # permuted into consumption order (k-half H0 of every rank first).
    xT_in = nc.dram_tensor("xT", [IN_DIM, nloc], f32, kind="ExternalInput")
    adjT_in = nc.dram_tensor("adjT", [n, nloc], f32, kind="ExternalInput")
    in_W = nc.dram_tensor("in_W", [IN_DIM, HID], f32, kind="ExternalInput")
    in_Wl = nc.dram_tensor("in_Wl", [IN_DIM, HID], f32, kind="ExternalInput")
    in_b = nc.dram_tensor("in_b", [HID], f32, kind="ExternalInput")
    blk_W1 = nc.dram_tensor("blk_W1", [L, HID, HID], f32, kind="ExternalInput")
    blk_Wl1 = nc.dram_tensor("blk_Wl1", [L, HID, HID], f32, kind="ExternalInput")
    blk_b1 = nc.dram_tensor("blk_b1", [L, HID], f32, kind="ExternalInput")
    blk_W2 = nc.dram_tensor("blk_W2", [L, HID, HID], f32, kind="ExternalInput")
    blk_Wl2 = nc.dram_tensor("blk_Wl2", [L, HID, HID], f32, kind="ExternalInput")
    blk_b2 = nc.dram_tensor("blk_b2", [L, HID], f32, kind="ExternalInput")
    out_W = nc.dram_tensor("out_W", [HID, OUT_DIM], f32, kind="ExternalInput")
    out_Wl = nc.dram_tensor("out_Wl", [HID, OUT_DIM], f32, kind="ExternalInput")
    out_b = nc.dram_tensor("out_b", [OUT_DIM], f32, kind="ExternalInput")
    x_out = nc.dram_tensor("x_out", [nloc, OUT_DIM], f32, kind="ExternalOutput")
    h_out = nc.dram_tensor("h_out", [nloc, HID], f32, kind="ExternalOutput")

    groups = [list(range(R))]

    with tile.TileContext(nc) as tc:
        # ------------- internal DRAM -------------
        from contextlib import ExitStack
        pools = ExitStack()
        dram_pool = pools.enter_context(tc.tile_pool(name="dram", bufs=1, space="DRAM"))

        def dram_tile(shape, dtype, name, **kw):
            return dram_pool.tile(shape, dtype, name=name, tag=name, **kw)

        NG = 2 * L + 1
        nh = nloc // 2   # rows per j-half
        s_ins = [(dram_tile([nh, HID], bf16, f"s_in_a{i}"),
                  dram_tile([nh, HID], bf16, f"s_in_b{i}"))
                 for i in range(NG)]
        s_fulls = [(dram_tile([R * nh, HID], bf16, f"s_full_a{i}",
                              addr_space="Shared"),
                    dram_tile([R * nh, HID], bf16, f"s_full_b{i}",
                              addr_space="Shared"))
                   for i in range(NG)]
        so_in = dram_tile([nloc, OUT_DIM], bf16, "so_in")
        so_full = dram_tile([n, OUT_DIM], bf16, "so_full", addr_space="Shared")

        # ------------- persistent SBUF -------------
        persist = pools.enter_context(tc.tile_pool(name="persist", bufs=1))

        def ptile(shape, dtype, name):
            return persist.tile(shape, dtype, name=name, tag=name)

        # transposed activations: fp32 carry + bf16 matmul copy, m-tiles 128/64
        ht32a = ptile([128, nloc], f32, "ht32a")
        ht32b = ptile([64, nloc], f32, "ht32b")
        tt32a = ptile([128, nloc], f32, "tt32a")
        tt32b = ptile([64, nloc], f32, "tt32b")
        htba = ptile([128, nloc], bf16, "htba")
        htbb = ptile([64, nloc], bf16, "htbb")
        # S_loc staging (j-major, ready for the AG bounce DMA)
        sloc = ptile([128, JT, HID], bf16, "sloc")
        soloc = ptile([128, JT, OUT_DIM], bf16, "soloc")

        # ------------- weights / biases to SBUF (bf16 / fp32) -------------
        def load_w(dram_ap, rows, name):
            """[rows, cols] fp32 DRAM -> list of bf16 SBUF tiles split at 128."""
            cols = dram_ap.shape[-1]
            tiles = []
            r0 = 0
            while r0 < rows:
                p = min(128, rows - r0)
                t = ptile([p, cols], bf16, f"{name}_{r0}")
                nc.gpsimd.dma_start(out=t[:, :], in_=dram_ap[r0:r0 + p, :])
                tiles.append(t)
                r0 += p
            return tiles

        def load_b(dram_ap, name):
            """[HID] fp32 DRAM -> ([128,1], [64,1]) fp32 SBUF tiles."""
            ta = ptile([128, 1], f32, f"{name}_a")
            tb = ptile([64, 1], f32, f"{name}_b")
            nc.sync.dma_start(out=ta[:, :], in_=dram_ap[0:128])
            nc.sync.dma_start(out=tb[:, :], in_=dram_ap[128:HID])
            return (ta, tb)

        in_W_sb = load_w(in_W.ap(), IN_DIM, "inW")
        in_Wl_sb = load_w(in_Wl.ap(), IN_DIM, "inWl")
        in_b_sb = load_b(in_b.ap(), "inb")
        Ws = []   # per hidden gconv: (W tiles, Wl tiles, bias pair)
        for l in range(L):
            Ws.append((load_w(blk_W1[l], HID, f"W1_{l}"),
                       load_w(blk_Wl1[l], HID, f"Wl1_{l}"),
                       load_b(blk_b1[l], f"b1_{l}")))
            Ws.append((load_w(blk_W2[l], HID, f"W2_{l}"),
                       load_w(blk_Wl2[l], HID, f"Wl2_{l}"),
                       load_b(blk_b2[l], f"b2_{l}")))
        outW_sb = load_w(out_W.ap(), HID, "outW")
        outWl_sb = load_w(out_Wl.ap(), HID, "outWl")
        outb_sb = ptile([OUT_DIM, 1], f32, "outb")
        nc.sync.dma_start(out=outb_sb[:, :], in_=out_b[0:OUT_DIM])

        iden = ptile([128, 128], f32, "iden")
        make_identity(nc, iden[:, :])

        # ------------- x (unblocks gconv 0): SWDGE cast-load -------------
        xt = ptile([128, IN_DIM // 128, nloc], bf16, "xt")
        for mt in range(IN_DIM // 128):
            nc.gpsimd.dma_start(out=xt[:, mt, :],
                                in_=xT_in[mt * 128:(mt + 1) * 128, :])
        xt_tiles = [xt[:, mt, :] for mt in range(IN_DIM // 128)]

        # ------------- PSUM pools -------------
        zpool = pools.enter_context(tc.tile_pool(name="zps", bufs=2, space="PSUM"))
        spool = pools.enter_context(tc.tile_pool(name="sps", bufs=2, space="PSUM"))
        schunk_pool = pools.enter_context(tc.tile_pool(name="schunk", bufs=2))

        M_SPLITS = [(0, 128), (128, HID)]  # m' column splits of the output

        JTH = JT // 2                    # j-tiles per half
        NH = 2                           # j halves
        KTH = KT // NH                   # k-tiles per half-AG
        CH2 = min(CH, KTH)
        JBS = [(jb * JB, JB) for jb in range(NJB)]   # j eviction regions

        def kt_of(h, i):
            """i-th k-tile of k-half h -> global k-tile index (AG output
            order is rank-major, each rank contributing its j-half rows)."""
            r, t = divmod(i, JTH)
            return r * JT + h * JTH + t

        # inverse: global k-tile -> index in the host-permuted adjT input
        inv_perm = [0] * KT
        for _h in range(NH):
            for _i in range(KTH):
                inv_perm[kt_of(_h, _i)] = _h * KTH + _i

        # ------------- adjT resident load -------------
        # adjT arrives host-transposed and tile-permuted; stream it in as
        # fp32 on the scalar HWDGE queue (keeping the sync queue free for
        # the gconv-critical DMAs) and cast to bf16 on DVE/ACT. Tiles land
        # in consumption order, so gconv 0 starts as soon as the first
        # k-half streams in.
        adjT_t = [ptile([128, nloc], bf16, f"adjT_{i}") for i in range(KT)]
        for i in range(KT):
            staged = schunk_pool.tile([128, nloc], f32, tag="s_ch",
                                      name=f"adj_stage_{i}")
            eng = nc.sync if (i < KTH and i % 2 == 0) else nc.scalar
            eng.dma_start(out=staged[:, :],
                          in_=adjT_in[i * 128:(i + 1) * 128, :])
            if i % 2 == 0:
                nc.vector.tensor_copy(adjT_t[i][:, :], staged[:, :])
            else:
                nc.scalar.activation(adjT_t[i][:, :], staged[:, :], AF.Copy)

        def sloc_mms(li, h, src_tiles, W_sb):
            """S_loc(li) rows for j-half h: matmuls, evict, bounce, AG."""
            for jt in range(h * JTH, (h + 1) * JTH):
                ps = spool.tile([128, HID], f32, tag="sps")
                for mi, src_t in enumerate(src_tiles):
                    nc.tensor.matmul(
                        ps[:, :],
                        lhsT=src_t[:, jt * 128:(jt + 1) * 128],
                        rhs=W_sb[mi][:, :],
                        start=(mi == 0),
                        stop=(mi == len(src_tiles) - 1),
                    )
                nc.vector.tensor_copy(sloc[:, jt, :], ps[:, :])
            nc.sync.dma_start(
                out=s_ins[li][h].rearrange("(t p) m -> p t m", p=128),
                in_=sloc[:, h * JTH:(h + 1) * JTH, :],
            )
            nc.gpsimd.collective_compute(
                "AllGather",
                mybir.AluOpType.bypass,
                replica_groups=groups,
                ins=[s_ins[li][h][:, :].opt()],
                outs=[s_fulls[li][h][:, :].opt()],
            )

        def quarter(li, zpair, h, jbs, stop_regions=(), reverse=False):
            """adj contraction MMs for k-half h over the j regions in jbs
            (each stationary load serves all regions in jbs back-to-back).
            stop_regions: j regions for which this call emits the final
            accumulation (stop=True on the LAST EMITTED k-tile).
            reverse: iterate chunks in reverse -- a re-fetching pass right
            after a forward pass starts on the chunk that is still
            resident in the pool slot."""
            sf = s_fulls[li][h].rearrange("(t p) m -> p t m", p=128)
            ncheck = KTH // CH2
            crange = range(ncheck - 1, -1, -1) if reverse else range(ncheck)
            for ci, c in enumerate(crange):
                s_ch = schunk_pool.tile([128, CH2, HID], bf16, tag="s_ch")
                nc.sync.dma_start(out=s_ch[:, :, :],
                                  in_=sf[:, c * CH2:(c + 1) * CH2, :])
                for q in range(CH2):
                    last_kt = ci == ncheck - 1 and q == CH2 - 1
                    for half, (m0, m1) in enumerate(M_SPLITS):
                        for (j0, jw) in jbs:
                            nc.tensor.matmul(
                                zpair[half][:, j0:j0 + jw],
                                lhsT=s_ch[:, q, m0:m1],
                                rhs=adjT_t[h * KTH + c * CH2 + q][:, j0:j0 + jw],
                                start=False,
                                stop=last_kt and (j0, jw) in stop_regions,
                            )

        def wl_mms(zpair, src_tiles, Wl_sb):
            for half, (m0, m1) in enumerate(M_SPLITS):
                for mi, src_t in enumerate(src_tiles):
                    for j0, jw in JBS:
                        nc.tensor.matmul(
                            zpair[half][:, j0:j0 + jw],
                            lhsT=Wl_sb[mi][:, m0:m1],
                            rhs=src_t[:, j0:j0 + jw],
                            start=(mi == 0),
                            stop=False,
                        )

        def evict(li, zpair, b_sb, j0, jw):
            """PSUM -> SBUF for j columns [j0, j0+jw): bias+relu f32 carry
            on ACT, plus bf16 matmul copy (fused on DVE, or via the
            residual arithmetic for second-of-block gconvs)."""
            is_b = li >= 2 and li % 2 == 0
            dst32 = (ht32a, ht32b) if li == 0 else (tt32a, tt32b)
            hb = (htba, htbb)
            for half in range(2):
                nc.scalar.activation(dst32[half][:, j0:j0 + jw],
                                     zpair[half][:, j0:j0 + jw],
                                     AF.Relu, bias=b_sb[half][:, :])
                if not is_b:
                    nc.vector.tensor_scalar(
                        hb[half][:, j0:j0 + jw], zpair[half][:, j0:j0 + jw],
                        scalar1=b_sb[half][:, :], scalar2=0.0,
                        op0=mybir.AluOpType.add, op1=mybir.AluOpType.max,
                    )
            if is_b:
                # h = (h + t) * 0.5 on this j range
                for hp, tp, hbt in ((ht32a, tt32a, htba), (ht32b, tt32b, htbb)):
                    nc.vector.tensor_add(hp[:, j0:j0 + jw], hp[:, j0:j0 + jw],
                                         tp[:, j0:j0 + jw])
                    nc.vector.tensor_scalar_mul(hbt[:, j0:j0 + jw],
                                                hp[:, j0:j0 + jw], 0.5)
                    nc.vector.tensor_scalar_mul(hp[:, j0:j0 + jw],
                                                hp[:, j0:j0 + jw], 0.5)

        h_tiles = [htba, htbb]
        NGC = 2 * L + 1   # relu gconvs (input + 12 block)

        def cfg(li):
            if li == 0:
                return (xt_tiles, in_W_sb, in_Wl_sb, in_b_sb)
            W, Wl, b = Ws[li - 1]
            return (h_tiles, W, Wl, b)

        # ---- pipelined gconv chain ----
        # gconv 0's support AGs come straight from x
        sloc_mms(0, 0, xt_tiles, in_W_sb)
        sloc_mms(0, 1, xt_tiles, in_W_sb)

        for li in range(NGC):
            src_t, W_sb, Wl_sb, b_sb = cfg(li)
            zps_a = zpool.tile([128, nloc], f32, tag="zps")
            zps_b = zpool.tile([64, nloc], f32, tag="zps")
            zpair = [zps_a, zps_b]
            wl_mms(zpair, src_t, Wl_sb)

            # H0 k-half over ALL j regions (one stationary load serves both
            # j blocks), then H1 over j-block 0 only -> j-block 0 complete
            # at the ~3/4 point: evict it and launch the next gconv's first
            # support AllGather while H1 x j-block 1 still computes.
            quarter(li, zpair, 0, JBS)
            quarter(li, zpair, 1, [JBS[0]], stop_regions=[JBS[0]])
            evict(li, zpair, b_sb, *JBS[0])
            if li + 1 < NGC:
                nsrc, nW, _, _ = cfg(li + 1)
                sloc_mms(li + 1, 0, nsrc, nW)

            if NJB == 2:
                quarter(li, zpair, 1, [JBS[1]], stop_regions=[JBS[1]],
                        reverse=True)
                evict(li, zpair, b_sb, *JBS[1])
            if li + 1 < NGC:
                nsrc, nW, _, _ = cfg(li + 1)
                sloc_mms(li + 1, 1, nsrc, nW)

        # ---- output gconv: x_out = adj @ (h@out_W) + h@out_Wl + out_b ----
        # S_out_loc = h @ out_W  [nloc, 3]
        for jt in range(JT):
            ps = spool.tile([128, OUT_DIM], f32, tag="sps")
            for mi, src in enumerate(h_tiles):
                nc.tensor.matmul(
                    ps[:, :],
                    lhsT=src[:, jt * 128:(jt + 1) * 128],
                    rhs=outW_sb[mi][:, :],
                    start=(mi == 0),
                    stop=(mi == 1),
                )
            nc.vector.tensor_copy(soloc[:, jt, :], ps[:, :])
        nc.sync.dma_start(
            out=so_in.rearrange("(t p) m -> p t m", p=128),
            in_=soloc[:, :, :],
        )
        nc.gpsimd.collective_compute(
            "AllGather",
            mybir.AluOpType.bypass,
            replica_groups=groups,
            ins=[so_in[:, :].opt()],
            outs=[so_full[:, :].opt()],
        )

        # ---- h_out: transpose H^T back to [nloc, HID] via TensorE ----
        hst_pool = pools.enter_context(tc.tile_pool(name="hst", bufs=2))
        tpool = pools.enter_context(tc.tile_pool(name="tps", bufs=2, space="PSUM"))
        for jt in range(JT):
            hstage = hst_pool.tile([128, HID], f32, tag="hst")
            tp_a = tpool.tile([128, 128], f32, tag="tps")
            nc.tensor.transpose(tp_a[:, :], ht32a[:, jt * 128:(jt + 1) * 128],
                                iden[:, :])
            nc.vector.tensor_copy(hstage[:, 0:128], tp_a[:, :])
            tp_b = tpool.tile([128, 64], f32, tag="tps")
            nc.tensor.transpose(tp_b[:, :], ht32b[:, jt * 128:(jt + 1) * 128],
                                iden[0:64, 0:64])
            nc.vector.tensor_copy(hstage[:, 128:HID], tp_b[:, :])
            nc.sync.dma_start(out=h_out[jt * 128:(jt + 1) * 128, :],
                              in_=hstage[:, :])

        xo_ps = zpool.tile([OUT_DIM, nloc], f32, tag="zps")
        # h @ out_Wl part (local, overlaps AG)
        for mi, src in enumerate(h_tiles):
            for jb in range(NJB):
                nc.tensor.matmul(
                    xo_ps[:, jb * JB:(jb + 1) * JB],
                    lhsT=outWl_sb[mi][:, :],
                    rhs=src[:, jb * JB:(jb + 1) * JB],
                    start=(mi == 0),
                    stop=False,
                )
        so_sb = ptile([128, KT, OUT_DIM], bf16, "so_sb")
        nc.sync.dma_start(
            out=so_sb[:, :, :],
            in_=so_full.rearrange("(t p) m -> p t m", p=128),
        )
        for kt in range(KT):
            for jb in range(NJB):
                nc.tensor.matmul(
                    xo_ps[:, jb * JB:(jb + 1) * JB],
                    lhsT=so_sb[:, kt, :],
                    rhs=adjT_t[inv_perm[kt]][:, jb * JB:(jb + 1) * JB],
                    start=False,
                    stop=(kt == KT - 1),
                )
        xo_sb = ptile([OUT_DIM, nloc], f32, "xo_sb")
        nc.vector.tensor_copy(xo_sb[:, :], xo_ps[:, :])
        nc.vector.tensor_scalar_add(xo_sb[:, :], xo_sb[:, :], outb_sb[:, :])
        nc.sync.dma_start(
            out=x_out.ap().rearrange("j m -> m j"),
            in_=xo_sb[:, :],
        )

        pools.close()

    nc.compile()
    return nc


def _shard_inputs(np_inputs, n):
    nloc = n // R
    KT, JT = n // 128, nloc // 128
    JTH = JT // 2
    # k-tile permutation: the j-half-0 tiles of every rank first (matches
    # the device's AllGather-half consumption order)
    perm = ([r * JT + t for r in range(R) for t in range(JTH)]
            + [r * JT + JTH + t for r in range(R) for t in range(JTH)])
    rep_keys = ["in_W", "in_Wl", "in_b", "blk_W1", "blk_Wl1", "blk_b1",
                "blk_W2", "blk_Wl2", "blk_b2", "out_W", "out_Wl", "out_b"]
    in_maps = []
    for i in range(R):
        adjT = np_inputs["adj"][i * nloc:(i + 1) * nloc].T  # [n, nloc]
        adjT = np.ascontiguousarray(
            adjT.reshape(KT, 128, nloc)[perm].reshape(n, nloc))
        m = {
            "xT": np.ascontiguousarray(
                np_inputs["x"][i * nloc:(i + 1) * nloc].T),
            "adjT": adjT,
        }
        for k in rep_keys:
            m[k] = np.ascontiguousarray(np_inputs[k])
        in_maps.append(m)
    return in_maps


def kernel(**inputs):
    global LAST_RESULT
    from concourse import bass_utils

    np_inputs = {k: np.ascontiguousarray(np.asarray(v, dtype=np.float32))
                 for k, v in inputs.items()}
    n = np_inputs["adj"].shape[0]

    if n not in _cache:
        _cache[n] = build_kernel(n=n)
    nc = _cache[n]

    in_maps = _shard_inputs(np_inputs, n)
    res = bass_utils.run_bass_kernel_spmd(
        nc, in_maps, core_ids=list(range(R)), trace=TRACE,
    )
    LAST_RESULT = res
    x_out = np.concatenate([res.results[i]["x_out"] for i in range(R)], axis=0)
    h = np.concatenate([res.results[i]["h_out"] for i in range(R)], axis=0)
    return x_out, h


if __name__ == "__main__":
    # quick CoreSim smoke test on a reduced problem (n=1024)
    from concourse.bass_interp import MultiCoreSim

    n_small = 2048
    nloc = n_small // R
    rng = np.random.default_rng(0)
    inp = {
        "x": rng.standard_normal((n_small, IN_DIM), dtype=np.float32),
        "adj": rng.random((n_small, n_small), dtype=np.float32),
        "in_W": (rng.standard_normal((IN_DIM, HID)) / np.sqrt(IN_DIM)).astype(np.float32),
        "in_Wl": (rng.standard_normal((IN_DIM, HID)) / np.sqrt(IN_DIM)).astype(np.float32),
        "in_b": np.zeros(HID, np.float32),
        "blk_W1": (rng.standard_normal((L, HID, HID)) / np.sqrt(HID)).astype(np.float32),
        "blk_Wl1": (rng.standard_normal((L, HID, HID)) / np.sqrt(HID)).astype(np.float32),
        "blk_b1": np.zeros((L, HID), np.float32),
        "blk_W2": (rng.standard_normal((L, HID, HID)) / np.sqrt(HID)).astype(np.float32),
        "blk_Wl2": (rng.standard_normal((L, HID, HID)) / np.sqrt(HID)).astype(np.float32),
        "blk_b2": np.zeros((L, HID), np.float32),
        "out_W": (rng.standard_normal((HID, OUT_DIM)) / np.sqrt(HID)).astype(np.float32),
        "out_Wl": (rng.standard_normal((HID, OUT_DIM)) / np.sqrt(HID)).astype(np.float32),
        "out_b": np.zeros(OUT_DIM, np.float32),
    }
    inp["adj"] = inp["adj"] / inp["adj"].sum(1, keepdims=True)

    # numpy reference
    def ref(i):
        def g(h, W, Wl, b):
            return i["adj"] @ (h @ W) + h @ Wl + b
        h = np.maximum(g(i["x"], i["in_W"], i["in_Wl"], i["in_b"]), 0)
        for l in range(L):
            t = np.maximum(g(h, i["blk_W1"][l], i["blk_Wl1"][l], i["blk_b1"][l]), 0)
            t = np.maximum(g(t, i["blk_W2"][l], i["blk_Wl2"][l], i["blk_b2"][l]), 0)
            h = (h + t) * 0.5
        return g(h, i["out_W"], i["out_Wl"], i["out_b"]), h

    exp_x, exp_h = ref(inp)

    nc = build_kernel(n=n_small)
    print("build + compile OK:", sum(len(f.instructions) for f in nc.m.functions if hasattr(f, 'instructions')) if hasattr(nc.m.functions[0], 'instructions') else "n/a")

    sim = MultiCoreSim(nc, R)
    in_maps = _shard_inputs(inp, n_small)
    for i in range(R):
        for k, v in in_maps[i].items():
            sim.cores[i].tensor(k)[:] = v
    sim.simulate(check_with_hw=False)

    got_x = np.concatenate([sim.cores[i].mem_tensor("x_out") for i in range(R)])
    got_h = np.concatenate([sim.cores[i].mem_tensor("h_out") for i in range(R)])
    for name, got, exp in (("x_out", got_x, exp_x), ("h", got_h, exp_h)):
        err = np.abs(got - exp).max() / np.abs(exp).max()
        print(f"{name}: rel absmax err = {err:.3e}")


# revision 28
# speedup vs baseline: 1.6487x; 1.6487x over previous
# Distributed Bass kernel for GBottleneck GNN (Pixel2Mesh GConv stack) on 8
# TRN2 NeuronCores.
#
# Reference computation (per layer):  Z = adj @ (h @ W) + h @ Wl + b
#   x: [8192, 256], adj: [8192, 8192], hidden 192, 6 residual blocks of 2
#   gconvs, final output conv to 3 dims. 14 gconvs total.
#
# Sharding: node dim N=8192 row-sharded across 8 cores (1024 rows each of
# adj/x/h). Weights replicated. Per gconv, each core computes its local
# support rows S_loc = h_loc @ W [1024, 192], all-gathers S (bf16), and
# contracts its adj rows against the full support.
#
# Key layout decisions:
#  - adj shard is cast fp32->bf16 once (SWDGE cast DMA, HBM->HBM), then
#    DMA-transposed (XBAR) into SBUF where it stays RESIDENT for all 14
#    gconvs: adjT [k=8192 (64 tiles of 128 partitions), j=1024] bf16, 16MB.
#    adj is read from HBM exactly once.
#  - Activations are kept transposed: Ht [192, 1024] (fp32 carry + bf16
#    matmul copy). In this orientation the adj contraction, the Wl path and
#    the bias+relu eviction all happen in PSUM [m', j] with zero extra
#    transposes; S_loc = h @ W comes out j-major which is exactly what the
#    all-gather concatenation needs.
#  - Big matmul: stationary = S_full[k_tile, m'] (bf16), moving = adjT
#    [k_tile, j], accumulating Z^T [192, 1024] in PSUM over 64 k tiles.
#    The h @ Wl path accumulates into the same PSUM banks first (it only
#    depends on local data, so it overlaps the AllGather latency).
#  - fp32 PSUM accumulation everywhere; residual carry in fp32.

import os
import sys

import numpy as np

if "/opt/trn_rl_repo" not in sys.path:
    sys.path.insert(0, "/opt/trn_rl_repo")

N, IN_DIM, HID, OUT_DIM, L = 8192, 256, 192, 3, 6
R = 8  # cores
# adj values (~1/N after row normalization) sit in fp8e4m3's subnormal
# range; pre-scaling by 4096 moves them to ~U(0,2). The inverse is folded
# into the PSUM evictions (and the h@Wl weights are pre-scaled to match).
ADJ_SCALE = 4096.0

TRACE = False  # test.py sets this for profiled runs
LAST_RESULT = None  # BassKernelResults of the most recent run

_cache = {}


def build_kernel(n=N, debug=False):
    import concourse.bass as bass
    import concourse.mybir as mybir
    import concourse.tile as tile
    from concourse import bacc
    from concourse.masks import make_identity

    f32 = mybir.dt.float32
    bf16 = mybir.dt.bfloat16
    f8 = mybir.dt.float8e4
    AF = mybir.ActivationFunctionType

    nloc = n // R
    KT = n // 128          # global k tiles
    JT = nloc // 128       # local j tiles
    assert JT % 4 == 0, "need JT divisible by 4 (even j-tiles per half)"
    JB = 512 if nloc >= 512 else nloc  # moving-operand j block
    NJB = nloc // JB
    CH = min(16, KT)       # k-tiles per S-chunk DMA
    NCH = KT // CH
    ACH = min(4, KT)       # k-tiles per adj cast chunk (column chunk)

    nc = bacc.Bacc("TRN2", target_bir_lowering=False, debug=debug, num_devices=R)

    # ---------------- I/O ----------------
    # x and adj arrive HOST-TRANSPOSED (layout prep done in kernel() while
    # sharding): xT = x_loc.T [IN_DIM, nloc]; adjT = adj_loc.T with k-tiles
    # permuted into consumption order (k-half H0 of every rank first).
    xT_in = nc.dram_tensor("xT", [IN_DIM, nloc], f32, kind="ExternalInput")
    adjT_in = nc.dram_tensor("adjT", [n, nloc], f8, kind="ExternalInput")
    in_W = nc.dram_tensor("in_W", [IN_DIM, HID], f32, kind="ExternalInput")
    in_Wl = nc.dram_tensor("in_Wl", [IN_DIM, HID], f32, kind="ExternalInput")
    in_b = nc.dram_tensor("in_b", [HID], f32, kind="ExternalInput")
    blk_W1 = nc.dram_tensor("blk_W1", [L, HID, HID], f32, kind="ExternalInput")
    blk_Wl1 = nc.dram_tensor("blk_Wl1", [L, HID, HID], f32, kind="ExternalInput")
    blk_b1 = nc.dram_tensor("blk_b1", [L, HID], f32, kind="ExternalInput")
    blk_W2 = nc.dram_tensor("blk_W2", [L, HID, HID], f32, kind="ExternalInput")
    blk_Wl2 = nc.dram_tensor("blk_Wl2", [L, HID, HID], f32, kind="ExternalInput")
    blk_b2 = nc.dram_tensor("blk_b2", [L, HID], f32, kind="ExternalInput")
    out_W = nc.dram_tensor("out_W", [HID, OUT_DIM], f32, kind="ExternalInput")
    out_Wl = nc.dram_tensor("out_Wl", [HID, OUT_DIM], f32, kind="ExternalInput")
    out_b = nc.dram_tensor("out_b", [OUT_DIM], f32, kind="ExternalInput")
    x_out = nc.dram_tensor("x_out", [nloc, OUT_DIM], f32, kind="ExternalOutput")
    h_out = nc.dram_tensor("h_out", [nloc, HID], f32, kind="ExternalOutput")

    groups = [list(range(R))]

    with tile.TileContext(nc) as tc:
        # ------------- internal DRAM -------------
        from contextlib import ExitStack
        pools = ExitStack()
        dram_pool = pools.enter_context(tc.tile_pool(name="dram", bufs=1, space="DRAM"))

        def dram_tile(shape, dtype, name, **kw):
            return dram_pool.tile(shape, dtype, name=name, tag=name, **kw)

        NG = 2 * L + 1
        nh = nloc // 2   # rows per j-half
        s_ins = [(dram_tile([nh, HID], f8, f"s_in_a{i}"),
                  dram_tile([nh, HID], f8, f"s_in_b{i}"))
                 for i in range(NG)]
        s_fulls = [(dram_tile([R * nh, HID], f8, f"s_full_a{i}",
                              addr_space="Shared"),
                    dram_tile([R * nh, HID], f8, f"s_full_b{i}",
                              addr_space="Shared"))
                   for i in range(NG)]
        so_in = dram_tile([nloc, OUT_DIM], f8, "so_in")
        so_full = dram_tile([n, OUT_DIM], f8, "so_full", addr_space="Shared")

        # ------------- persistent SBUF -------------
        persist = pools.enter_context(tc.tile_pool(name="persist", bufs=1))

        def ptile(shape, dtype, name):
            return persist.tile(shape, dtype, name=name, tag=name)

        # transposed activations: fp32 carry + bf16 matmul copy, m-tiles 128/64
        ht32a = ptile([128, nloc], f32, "ht32a")
        ht32b = ptile([64, nloc], f32, "ht32b")
        tt32a = ptile([128, nloc], f32, "tt32a")
        tt32b = ptile([64, nloc], f32, "tt32b")
        htba = ptile([128, nloc], bf16, "htba")
        htbb = ptile([64, nloc], bf16, "htbb")
        # S_loc staging (j-major, ready for the AG bounce DMA)
        sloc = ptile([128, JT, HID], f8, "sloc")
        soloc = ptile([128, JT, OUT_DIM], f8, "soloc")

        # ------------- weights / biases to SBUF (bf16 / fp32) -------------
        def load_w(dram_ap, rows, name):
            """[rows, cols] fp32 DRAM -> list of bf16 SBUF tiles split at 128."""
            cols = dram_ap.shape[-1]
            tiles = []
            r0 = 0
            while r0 < rows:
                p = min(128, rows - r0)
                t = ptile([p, cols], bf16, f"{name}_{r0}")
                nc.gpsimd.dma_start(out=t[:, :], in_=dram_ap[r0:r0 + p, :])
                tiles.append(t)
                r0 += p
            return tiles

        def load_b(dram_ap, name):
            """[HID] fp32 DRAM -> ([128,1], [64,1]) fp32 SBUF tiles."""
            ta = ptile([128, 1], f32, f"{name}_a")
            tb = ptile([64, 1], f32, f"{name}_b")
            nc.sync.dma_start(out=ta[:, :], in_=dram_ap[0:128])
            nc.sync.dma_start(out=tb[:, :], in_=dram_ap[128:HID])
            return (ta, tb)

        in_W_sb = load_w(in_W.ap(), IN_DIM, "inW")
        in_Wl_sb = load_w(in_Wl.ap(), IN_DIM, "inWl")
        in_b_sb = load_b(in_b.ap(), "inb")
        Ws = []   # per hidden gconv: (W tiles, Wl tiles, bias pair)
        for l in range(L):
            Ws.append((load_w(blk_W1[l], HID, f"W1_{l}"),
                       load_w(blk_Wl1[l], HID, f"Wl1_{l}"),
                       load_b(blk_b1[l], f"b1_{l}")))
            Ws.append((load_w(blk_W2[l], HID, f"W2_{l}"),
                       load_w(blk_Wl2[l], HID, f"Wl2_{l}"),
                       load_b(blk_b2[l], f"b2_{l}")))
        outW_sb = load_w(out_W.ap(), HID, "outW")
        outWl_sb = load_w(out_Wl.ap(), HID, "outWl")
        outb_sb = ptile([OUT_DIM, 1], f32, "outb")
        nc.sync.dma_start(out=outb_sb[:, :], in_=out_b[0:OUT_DIM])

        iden = ptile([128, 128], f32, "iden")
        make_identity(nc, iden[:, :])

        # ------------- x (unblocks gconv 0): SWDGE cast-load -------------
        xt = ptile([128, IN_DIM // 128, nloc], bf16, "xt")
        for mt in range(IN_DIM // 128):
            nc.gpsimd.dma_start(out=xt[:, mt, :],
                                in_=xT_in[mt * 128:(mt + 1) * 128, :])
        xt_tiles = [xt[:, mt, :] for mt in range(IN_DIM // 128)]

        # ------------- PSUM pools -------------
        zpool = pools.enter_context(tc.tile_pool(name="zps", bufs=2, space="PSUM"))
        spool = pools.enter_context(tc.tile_pool(name="sps", bufs=2, space="PSUM"))
        schunk_pool = pools.enter_context(tc.tile_pool(name="schunk", bufs=2))

        M_SPLITS = [(0, 128), (128, HID)]  # m' column splits of the output

        JTH = JT // 2                    # j-tiles per half
        NH = 2                           # j halves
        KTH = KT // NH                   # k-tiles per half-AG
        CH2 = min(CH, KTH)
        JBS = [(jb * JB, JB) for jb in range(NJB)]   # j eviction regions

        def kt_of(h, i):
            """i-th k-tile of k-half h -> global k-tile index (AG output
            order is rank-major, each rank contributing its j-half rows)."""
            r, t = divmod(i, JTH)
            return r * JT + h * JTH + t

        # inverse: global k-tile -> index in the host-permuted adjT input
        inv_perm = [0] * KT
        for _h in range(NH):
            for _i in range(KTH):
                inv_perm[kt_of(_h, _i)] = _h * KTH + _i

        # ------------- adjT resident load -------------
        # adjT arrives host-transposed, tile-permuted, pre-scaled by
        # ADJ_SCALE and cast to fp8e4m3 -- so it DMAs straight into the
        # resident SBUF pair-tiles [128, 2, nloc] (DoubleRow layout:
        # element (p, e, j) is adj column k = pair_base + e*128 + p).
        NPAIR = KT // 2
        adjT_p = [ptile([128, 2, nloc], f8, f"adjT_{i}") for i in range(NPAIR)]
        for i in range(NPAIR):
            nc.scalar.dma_start(
                out=adjT_p[i][:, :, :],
                in_=adjT_in[2 * i * 128:(2 * i + 2) * 128, :].rearrange(
                    "(e p) m -> p e m", p=128),
            )

        def sloc_mms(li, h, src_tiles, W_sb):
            """S_loc(li) rows for j-half h: matmuls, evict, bounce, AG."""
            for jt in range(h * JTH, (h + 1) * JTH):
                ps = spool.tile([128, HID], f32, tag="sps")
                for mi, src_t in enumerate(src_tiles):
                    nc.tensor.matmul(
                        ps[:, :],
                        lhsT=src_t[:, jt * 128:(jt + 1) * 128],
                        rhs=W_sb[mi][:, :],
                        start=(mi == 0),
                        stop=(mi == len(src_tiles) - 1),
                    )
                nc.vector.tensor_copy(sloc[:, jt, :], ps[:, :])
            nc.sync.dma_start(
                out=s_ins[li][h].rearrange("(t p) m -> p t m", p=128),
                in_=sloc[:, h * JTH:(h + 1) * JTH, :],
            )
            nc.gpsimd.collective_compute(
                "AllGather",
                mybir.AluOpType.bypass,
                replica_groups=groups,
                ins=[s_ins[li][h][:, :].opt()],
                outs=[s_fulls[li][h][:, :].opt()],
            )

        def quarter(li, zpair, h, jbs, stop_regions=(), reverse=False):
            """adj contraction MMs for k-half h over the j regions in jbs
            (each stationary load serves all regions in jbs back-to-back).
            stop_regions: j regions for which this call emits the final
            accumulation (stop=True on the LAST EMITTED k-tile).
            reverse: iterate chunks in reverse -- a re-fetching pass right
            after a forward pass starts on the chunk that is still
            resident in the pool slot."""
            sf = s_fulls[li][h].rearrange("(t p) m -> p t m", p=128)
            ncheck = KTH // CH2
            crange = range(ncheck - 1, -1, -1) if reverse else range(ncheck)
            for ci, c in enumerate(crange):
                s_ch = schunk_pool.tile([128, CH2, HID], f8, tag="s_ch")
                nc.sync.dma_start(out=s_ch[:, :, :],
                                  in_=sf[:, c * CH2:(c + 1) * CH2, :])
                for q in range(0, CH2, 2):
                    last_kt = ci == ncheck - 1 and q == CH2 - 2
                    pair = (h * KTH + c * CH2 + q) // 2
                    for half, (m0, m1) in enumerate(M_SPLITS):
                        for (j0, jw) in jbs:
                            nc.tensor.matmul(
                                zpair[half][:, j0:j0 + jw],
                                lhsT=s_ch[:, q:q + 2, m0:m1],
                                rhs=adjT_p[pair][:, :, j0:j0 + jw],
                                start=False,
                                stop=last_kt and (j0, jw) in stop_regions,
                                perf_mode=mybir.MatmulPerfMode.DoubleRow,
                            )

        def wl_mms(zpair, src_tiles, Wl_sb):
            for half, (m0, m1) in enumerate(M_SPLITS):
                for mi, src_t in enumerate(src_tiles):
                    for j0, jw in JBS:
                        nc.tensor.matmul(
                            zpair[half][:, j0:j0 + jw],
                            lhsT=Wl_sb[mi][:, m0:m1],
                            rhs=src_t[:, j0:j0 + jw],
                            start=(mi == 0),
                            stop=False,
                        )

        def evict(li, zpair, b_sb, j0, jw):
            """PSUM -> SBUF for j columns [j0, j0+jw): bias+relu f32 carry
            on ACT, plus bf16 matmul copy (fused on DVE, or via the
            residual arithmetic for second-of-block gconvs)."""
            is_b = li >= 2 and li % 2 == 0
            dst32 = (ht32a, ht32b) if li == 0 else (tt32a, tt32b)
            hb = (htba, htbb)
            for half in range(2):
                nc.scalar.activation(dst32[half][:, j0:j0 + jw],
                                     zpair[half][:, j0:j0 + jw],
                                     AF.Relu, bias=b_sb[half][:, :],
                                     scale=1.0 / ADJ_SCALE)
                if not is_b:
                    nc.vector.tensor_copy(hb[half][:, j0:j0 + jw],
                                          dst32[half][:, j0:j0 + jw])
            if is_b:
                # h = (h + t) * 0.5 on this j range
                for hp, tp, hbt in ((ht32a, tt32a, htba), (ht32b, tt32b, htbb)):
                    nc.vector.tensor_add(hp[:, j0:j0 + jw], hp[:, j0:j0 + jw],
                                         tp[:, j0:j0 + jw])
                    nc.vector.tensor_scalar_mul(hbt[:, j0:j0 + jw],
                                                hp[:, j0:j0 + jw], 0.5)
                    nc.vector.tensor_scalar_mul(hp[:, j0:j0 + jw],
                                                hp[:, j0:j0 + jw], 0.5)

        h_tiles = [htba, htbb]
        NGC = 2 * L + 1   # relu gconvs (input + 12 block)

        def cfg(li):
            if li == 0:
                return (xt_tiles, in_W_sb, in_Wl_sb, in_b_sb)
            W, Wl, b = Ws[li - 1]
            return (h_tiles, W, Wl, b)

        # ---- pipelined gconv chain ----
        # gconv 0's support AGs come straight from x
        sloc_mms(0, 0, xt_tiles, in_W_sb)
        sloc_mms(0, 1, xt_tiles, in_W_sb)

        for li in range(NGC):
            src_t, W_sb, Wl_sb, b_sb = cfg(li)
            zps_a = zpool.tile([128, nloc], f32, tag="zps")
            zps_b = zpool.tile([64, nloc], f32, tag="zps")
            zpair = [zps_a, zps_b]
            wl_mms(zpair, src_t, Wl_sb)

            # H0 k-half over ALL j regions (one stationary load serves both
            # j blocks), then H1 over j-block 0 only -> j-block 0 complete
            # at the ~3/4 point: evict it and launch the next gconv's first
            # support AllGather while H1 x j-block 1 still computes.
            quarter(li, zpair, 0, JBS)
            quarter(li, zpair, 1, [JBS[0]], stop_regions=[JBS[0]])
            evict(li, zpair, b_sb, *JBS[0])
            if li + 1 < NGC:
                nsrc, nW, _, _ = cfg(li + 1)
                sloc_mms(li + 1, 0, nsrc, nW)

            if NJB == 2:
                quarter(li, zpair, 1, [JBS[1]], stop_regions=[JBS[1]],
                        reverse=True)
                evict(li, zpair, b_sb, *JBS[1])
            if li + 1 < NGC:
                nsrc, nW, _, _ = cfg(li + 1)
                sloc_mms(li + 1, 1, nsrc, nW)

        # ---- output gconv: x_out = adj @ (h@out_W) + h@out_Wl + out_b ----
        # S_out_loc = h @ out_W  [nloc, 3]
        for jt in range(JT):
            ps = spool.tile([128, OUT_DIM], f32, tag="sps")
            for mi, src in enumerate(h_tiles):
                nc.tensor.matmul(
                    ps[:, :],
                    lhsT=src[:, jt * 128:(jt + 1) * 128],
                    rhs=outW_sb[mi][:, :],
                    start=(mi == 0),
                    stop=(mi == 1),
                )
            nc.vector.tensor_copy(soloc[:, jt, :], ps[:, :])
        nc.sync.dma_start(
            out=so_in.rearrange("(t p) m -> p t m", p=128),
            in_=soloc[:, :, :],
        )
        nc.gpsimd.collective_compute(
            "AllGather",
            mybir.AluOpType.bypass,
            replica_groups=groups,
            ins=[so_in[:, :].opt()],
            outs=[so_full[:, :].opt()],
        )

        # ---- h_out: transpose H^T back to [nloc, HID] via TensorE ----
        hst_pool = pools.enter_context(tc.tile_pool(name="hst", bufs=2))
        tpool = pools.enter_context(tc.tile_pool(name="tps", bufs=2, space="PSUM"))
        for jt in range(JT):
            hstage = hst_pool.tile([128, HID], f32, tag="hst")
            tp_a = tpool.tile([128, 128], f32, tag="tps")
            nc.tensor.transpose(tp_a[:, :], ht32a[:, jt * 128:(jt + 1) * 128],
                                iden[:, :])
            nc.vector.tensor_copy(hstage[:, 0:128], tp_a[:, :])
            tp_b = tpool.tile([128, 64], f32, tag="tps")
            nc.tensor.transpose(tp_b[:, :], ht32b[:, jt * 128:(jt + 1) * 128],
                                iden[0:64, 0:64])
            nc.vector.tensor_copy(hstage[:, 128:HID], tp_b[:, :])
            nc.sync.dma_start(out=h_out[jt * 128:(jt + 1) * 128, :],
                              in_=hstage[:, :])

        xo_ps = zpool.tile([OUT_DIM, nloc], f32, tag="zps")
        # h @ out_Wl part (local, overlaps AG)
        for mi, src in enumerate(h_tiles):
            for jb in range(NJB):
                nc.tensor.matmul(
                    xo_ps[:, jb * JB:(jb + 1) * JB],
                    lhsT=outWl_sb[mi][:, :],
                    rhs=src[:, jb * JB:(jb + 1) * JB],
                    start=(mi == 0),
                    stop=False,
                )
        # stage S_out in the PERMUTED k order so pair u lines up with
        # adjT_p[u]: global k-tile of permuted index i is kt_of-inverse.
        so_sb = ptile([128, KT, OUT_DIM], f8, "so_sb")
        sof = so_full.rearrange("(t p) m -> p t m", p=128)
        perm_kt = [kt_of(_h, _i) for _h in range(NH) for _i in range(KTH)]
        for i in range(0, KT, 2):
            # permuted-consecutive tiles are global-consecutive (pairs never
            # straddle an AG half), so one DMA covers the pair
            assert perm_kt[i + 1] == perm_kt[i] + 1
            nc.sync.dma_start(out=so_sb[:, i:i + 2, :],
                              in_=sof[:, perm_kt[i]:perm_kt[i] + 2, :])
        # plain fp8 matmuls here: DoubleRow needs the k-pair stride to be
        # 16B-aligned and S_out is only 3 columns wide
        for u in range(NPAIR):
            for e in range(2):
                for jb in range(NJB):
                    nc.tensor.matmul(
                        xo_ps[:, jb * JB:(jb + 1) * JB],
                        lhsT=so_sb[:, 2 * u + e, :],
                        rhs=adjT_p[u][:, e, jb * JB:(jb + 1) * JB],
                        start=False,
                        stop=(u == NPAIR - 1 and e == 1),
                    )
        xo_sb = ptile([OUT_DIM, nloc], f32, "xo_sb")
        nc.vector.tensor_scalar(xo_sb[:, :], xo_ps[:, :],
                                scalar1=1.0 / ADJ_SCALE,
                                scalar2=outb_sb[:, :],
                                op0=mybir.AluOpType.mult,
                                op1=mybir.AluOpType.add)
        nc.sync.dma_start(
            out=x_out.ap().rearrange("j m -> m j"),
            in_=xo_sb[:, :],
        )

        pools.close()

    nc.compile()
    return nc


def _shard_inputs(np_inputs, n):
    nloc = n // R
    KT, JT = n // 128, nloc // 128
    JTH = JT // 2
    # k-tile permutation: the j-half-0 tiles of every rank first (matches
    # the device's AllGather-half consumption order)
    perm = ([r * JT + t for r in range(R) for t in range(JTH)]
            + [r * JT + JTH + t for r in range(R) for t in range(JTH)])
    rep_keys = ["in_W", "in_Wl", "in_b", "blk_W1", "blk_Wl1", "blk_b1",
                "blk_W2", "blk_Wl2", "blk_b2", "out_W", "out_Wl", "out_b"]
    import ml_dtypes

    # the h@Wl path accumulates into the same PSUM as the (pre-scaled) adj
    # contraction, so its weights carry the same scale; the 1/ADJ_SCALE is
    # applied once at eviction. Biases stay unscaled (added post-scale).
    scaled = dict(np_inputs)
    for k in ("in_Wl", "blk_Wl1", "blk_Wl2", "out_Wl"):
        scaled[k] = np_inputs[k] * np.float32(ADJ_SCALE)

    in_maps = []
    for i in range(R):
        adjT = np_inputs["adj"][i * nloc:(i + 1) * nloc].T  # [n, nloc]
        adjT = np.ascontiguousarray(
            adjT.reshape(KT, 128, nloc)[perm].reshape(n, nloc))
        adjT = (adjT * np.float32(ADJ_SCALE)).astype(ml_dtypes.float8_e4m3)
        m = {
            "xT": np.ascontiguousarray(
                np_inputs["x"][i * nloc:(i + 1) * nloc].T),
            "adjT": adjT,
        }
        for k in rep_keys:
            m[k] = np.ascontiguousarray(scaled[k])
        in_maps.append(m)
    return in_maps


def kernel(**inputs):
    global LAST_RESULT
    from concourse import bass_utils

    np_inputs = {k: np.ascontiguousarray(np.asarray(v, dtype=np.float32))
                 for k, v in inputs.items()}
    n = np_inputs["adj"].shape[0]

    if n not in _cache:
        _cache[n] = build_kernel(n=n)
    nc = _cache[n]

    in_maps = _shard_inputs(np_inputs, n)
    res = bass_utils.run_bass_kernel_spmd(
        nc, in_maps, core_ids=list(range(R)), trace=TRACE,
    )
    LAST_RESULT = res
    x_out = np.concatenate([res.results[i]["x_out"] for i in range(R)], axis=0)
    h = np.concatenate([res.results[i]["h_out"] for i in range(R)], axis=0)
    return x_out, h


if __name__ == "__main__":
    # quick CoreSim smoke test on a reduced problem (n=1024)
    from concourse.bass_interp import MultiCoreSim

    n_small = 4096
    nloc = n_small // R
    rng = np.random.default_rng(0)
    inp = {
        "x": rng.standard_normal((n_small, IN_DIM), dtype=np.float32),
        "adj": rng.random((n_small, n_small), dtype=np.float32),
        "in_W": (rng.standard_normal((IN_DIM, HID)) / np.sqrt(IN_DIM)).astype(np.float32),
        "in_Wl": (rng.standard_normal((IN_DIM, HID)) / np.sqrt(IN_DIM)).astype(np.float32),
        "in_b": np.zeros(HID, np.float32),
        "blk_W1": (rng.standard_normal((L, HID, HID)) / np.sqrt(HID)).astype(np.float32),
        "blk_Wl1": (rng.standard_normal((L, HID, HID)) / np.sqrt(HID)).astype(np.float32),
        "blk_b1": np.zeros((L, HID), np.float32),
        "blk_W2": (rng.standard_normal((L, HID, HID)) / np.sqrt(HID)).astype(np.float32),
        "blk_Wl2": (rng.standard_normal((L, HID, HID)) / np.sqrt(HID)).astype(np.float32),
        "blk_b2": np.zeros((L, HID), np.float32),
        "out_W": (rng.standard_normal((HID, OUT_DIM)) / np.sqrt(HID)).astype(np.float32),
        "out_Wl": (rng.standard_normal((HID, OUT_DIM)) / np.sqrt(HID)).astype(np.float32),
        "out_b": np.zeros(OUT_DIM, np.float32),
    }
    inp["adj"] = inp["adj"] / inp["adj"].sum(1, keepdims=True)

    # numpy reference
    def ref(i):
        def g(h, W, Wl, b):
            return i["adj"] @ (h @ W) + h @ Wl + b
        h = np.maximum(g(i["x"], i["in_W"], i["in_Wl"], i["in_b"]), 0)
        for l in range(L):
            t = np.maximum(g(h, i["blk_W1"][l], i["blk_Wl1"][l], i["blk_b1"][l]), 0)
            t = np.maximum(g(t, i["blk_W2"][l], i["blk_Wl2"][l], i["blk_b2"][l]), 0)
            h = (h + t) * 0.5
        return g(h, i["out_W"], i["out_Wl"], i["out_b"]), h

    exp_x, exp_h = ref(inp)

    nc = build_kernel(n=n_small)
    print("build + compile OK:", sum(len(f.instructions) for f in nc.m.functions if hasattr(f, 'instructions')) if hasattr(nc.m.functions[0], 'instructions') else "n/a")

    sim = MultiCoreSim(nc, R)
    in_maps = _shard_inputs(inp, n_small)
    for i in range(R):
        for k, v in in_maps[i].items():
            sim.cores[i].tensor(k)[:] = v
    sim.simulate(check_with_hw=False)

    got_x = np.concatenate([sim.cores[i].mem_tensor("x_out") for i in range(R)])
    got_h = np.concatenate([sim.cores[i].mem_tensor("h_out") for i in range(R)])
    for name, got, exp in (("x_out", got_x, exp_x), ("h", got_h, exp_h)):
        err = np.abs(got - exp).max() / np.abs(exp).max()
        print(f"{name}: rel absmax err = {err:.3e}")
